# revision 1
# baseline (speedup 1.0000x reference)
"""Trainium2 Bass kernel for the GRU encoder-decoder model.

Model (see harness reference): B=1024, T=100, PRED=30, E=512, H=1024, IN=2.
  emb = tanh(obs @ We.T + be)                      (B,T,512)
  enc1 = GRU(emb), enc2 = GRU(enc1, h0=h_enc1)     hidden 1024
  out0 = enc2[:,-2] @ Wout.T + bout
  30-step autoregressive decoder with two GRU cells sharing one hidden.

Strategy: data-parallel over batch on 8 cores (128 rows/core).  All compute is
done in a feature-on-partition ("transposed") layout: SBUF tiles are
[128 partitions = feature dim chunk, free = (tile, batch)].  Weights are
pre-transposed on the host and used as the stationary matmul operand in bf16;
PSUM accumulates in fp32.  The time-invariant input projections (gx = x@Wih.T
+ biases) for both encoder GRUs are precomputed in large N=512 matmul phases
and spilled to DRAM as bf16; the sequential scans then only run the h@Whh.T
recurrence plus the gate math.  The decoder is fused (autoregressive).  The
dec2 cell reads the same vector (h1) for both input and hidden, so its r/z
input+hidden weights are summed on the host, saving a third of its matmuls.
"""

import numpy as np
import ml_dtypes

import concourse.bass as bass
import concourse.mybir as mybir
import concourse.tile as tile
from concourse import bacc
from concourse.bass_utils import run_bass_kernel_spmd

F32 = mybir.dt.float32
BF16 = mybir.dt.bfloat16
FP16 = mybir.dt.float16
AF = mybir.ActivationFunctionType
ALU = mybir.AluOpType

N_CORES = 8
B, T, PRED = 1024, 100, 30
E, H, IN = 512, 1024, 2
BL = B // N_CORES          # 128 batch rows per core
G = 3 * H                  # 3072 stacked gate rows
KH = H // 128              # 8 hidden k-tiles
KE = E // 128              # 4 embedding k-tiles
NT = T * BL // 512         # 25 n-chunks of 512 in the gx phases
CHUNKS = [(0, 2), (2, 4), (4, 6), (6, 8)]  # h-tile chunks for the gate chain

_CACHE = {}


# ----------------------------------------------------------------------------
# device program
# ----------------------------------------------------------------------------

def _emit_gx_phase(nc, tc, wk, nk, rhs_fn, gx_dram, gxb_s, pools):
    """One n-chunk loop computing gx' = Wih.T-tiles @ rhs (+bias) -> DRAM.

    wk: SBUF weight tile [128, nk, G]; rhs_fn(c, pools) -> rhs tile [128, nk, 512]
    """
    ps_gx = pools["ps_gx"]
    gxop = pools["gxo"]
    for c in range(NT):
        rhs = rhs_fn(c)   # list of [128, 512] APs, one per k-tile
        for a in range(24):
            pg = ps_gx.tile([128, 512], F32, tag="pgx")
            for j in range(nk):
                nc.tensor.matmul(
                    pg[:], wk[:, j, a * 128:(a + 1) * 128], rhs[j],
                    start=(j == 0), stop=(j == nk - 1))
            gxo = gxop.tile([128, 4, 128], F32, tag="gxo")
            nc.scalar.activation(
                gxo.rearrange("p t b -> p (t b)"), pg[:], AF.Identity,
                bias=gxb_s[:, a:a + 1])
            nc.sync.dma_start(
                out=gx_dram[4 * c:4 * c + 4, a].rearrange("t p b -> p t b"),
                in_=gxo[:])


def _emit_out_block(nc, wout_s, bout_s, hb, ps_pool, ps_tag, outp,
                    preds=None, t=None):
    """outT = h @ Wout.T + bout -> ([2,128] f32, [2,128] bf16)."""
    po = ps_pool.tile([2, 128], F32, tag=ps_tag)
    for j in range(KH):
        nc.tensor.matmul(po[:], wout_s[:, j, :], hb[:, j, :],
                         start=(j == 0), stop=(j == KH - 1))
    outf = outp.tile([2, 128], F32, tag="outf")
    nc.vector.tensor_scalar_add(outf[:], po[:], bout_s[:, 0:1])
    outb = outp.tile([2, 128], FP16, tag="outb")
    nc.vector.tensor_copy(outb[:], outf[:])
    if preds is not None:
        nc.sync.dma_start(out=preds[:, t, :], in_=outf[:])
    return outf, outb


def _emit_chain(nc, pools, p_rz, p_ghn, hb, rz_sb, gxn_src, bhn_s, bin_s,
                perm=False):
    """Gate math after the matmuls: returns the new fp16 hidden state.

    rz_sb: [128,16,128] fp16 sigmoid(r,z).  With perm=True the r/z tiles are
    bank-interleaved [r0 r1 z0 z1 | r2 r3 z2 z3 | ...]; otherwise [r*8, z*8].
    gxn_src: either ("sbuf", gx_tile) with n-part at tiles 16:24 (biases
    folded) or ("psum", p_gxn) requiring the bin_s bias.
    """
    tmp = pools["tmp"]
    hbp = pools["hb"]
    t1 = tmp.tile([128, 8, 128], FP16, tag="t1")
    t2 = tmp.tile([128, 8, 128], FP16, tag="t2")
    nn = tmp.tile([128, 8, 128], FP16, tag="nn")
    d = tmp.tile([128, 8, 128], FP16, tag="d")
    hz = tmp.tile([128, 8, 128], FP16, tag="hz")
    nhb = hbp.tile([128, 8, 128], FP16, tag="hb")
    kind, gxn = gxn_src
    for lo, hi in CHUNKS:
        for j in range(lo, hi):
            rj = 4 * (j // 2) + (j % 2) if perm else j
            nc.vector.scalar_tensor_tensor(
                t1[:, j, :], p_ghn[:, j, :], bhn_s[:, j:j + 1], rz_sb[:, rj, :],
                op0=ALU.add, op1=ALU.mult)
            if kind == "sbuf":
                nc.vector.tensor_add(t2[:, j, :], t1[:, j, :], gxn[:, 16 + j, :])
            else:
                nc.vector.scalar_tensor_tensor(
                    t2[:, j, :], gxn[:, j, :], bin_s[:, j:j + 1], t1[:, j, :],
                    op0=ALU.add, op1=ALU.add)
        sl = slice(lo, hi)
        zsl = slice(2 * lo + 2, 2 * lo + 4) if perm else slice(8 + lo, 8 + hi)
        nc.scalar.activation(nn[:, sl, :], t2[:, sl, :], AF.Tanh)
        nc.vector.tensor_sub(d[:, sl, :], hb[:, sl, :], nn[:, sl, :])
        nc.vector.tensor_mul(hz[:, sl, :], d[:, sl, :], rz_sb[:, zsl, :])
        nc.vector.tensor_add(nhb[:, sl, :], hz[:, sl, :], nn[:, sl, :])
    return nhb


def _emit_enc_scan(nc, tc, pools, whh_s, gx_dram, bhn_s, hb,
                   ys_dram=None, out98=None):
    """Encoder scan over T steps.  Returns (hf, hb) and optionally (outf, outb)
    captured at t = T-2 via out98 = (wout_s, bout_s, outp)."""
    gxp = pools["gx"]
    tmp = pools["tmp"]
    ps_rz, ps_ghn = pools["ps_rz"], pools["ps_ghn"]
    PF = 4
    pend = []
    for t in range(min(PF, T)):
        gxt = gxp.tile([128, 24, 128], F32, tag="gxt")
        nc.sync.dma_start(out=gxt[:], in_=gx_dram[t].rearrange("a p b -> p a b"))
        pend.append(gxt)
    out_res = None
    for t in range(T):
        gxt = pend.pop(0)
        p_rz = ps_rz.tile([128, 16, 128], F32, tag="prz")
        rzs = tmp.tile([128, 16, 128], F32, tag="rzs")
        rz_sb = tmp.tile([128, 16, 128], FP16, tag="rz")
        # Bank-outer sweep: each PSUM bank (4 r/z tiles, bank-interleaved
        # [r0 r1 z0 z1 | ...]) is one accumulation group (start clears the
        # whole bank), and its bias-add + sigmoid issue as soon as the bank
        # finishes, overlapping the remaining banks' matmuls.
        for bk in range(4):
            for j in range(KH):           # k-inner: consume h chunks early
                for ai in range(4):
                    a = 4 * bk + ai
                    nc.tensor.matmul(
                        p_rz[:, a, :], whh_s[:, j, a * 128:(a + 1) * 128],
                        hb[:, j, :], start=(j == 0 and ai == 0),
                        stop=(j == KH - 1 and ai == 3))
            sl = slice(4 * bk, 4 * bk + 4)
            nc.vector.tensor_add(rzs[:, sl, :], p_rz[:, sl, :], gxt[:, sl, :])
            nc.scalar.activation(rz_sb[:, sl, :], rzs[:, sl, :], AF.Sigmoid)
        p_ghn = ps_ghn.tile([128, 8, 128], F32, tag="pghn")
        for a in range(8):                # m-outer: complete tiles early
            for j in range(KH):
                nc.tensor.matmul(
                    p_ghn[:, a, :], whh_s[:, j, (16 + a) * 128:(17 + a) * 128],
                    hb[:, j, :], start=(a % 4 == 0 and j == 0),
                    stop=(a % 4 == 3 and j == KH - 1))
        hb = _emit_chain(nc, pools, p_rz, p_ghn, hb, rz_sb,
                         ("sbuf", gxt), bhn_s, None, perm=True)
        if ys_dram is not None:
            nc.sync.dma_start(
                out=ys_dram[t].rearrange("j p b -> p j b"), in_=hb[:])
        if out98 is not None and t == T - 2:
            wout_s, bout_s, outp = out98
            out_res = _emit_out_block(nc, wout_s, bout_s, hb, ps_ghn, "pghn",
                                      outp)
        if t + PF < T:
            gxt2 = gxp.tile([128, 24, 128], F32, tag="gxt")
            nc.sync.dma_start(out=gxt2[:],
                              in_=gx_dram[t + PF].rearrange("a p b -> p a b"))
            pend.append(gxt2)
    return hb, out_res


def _emit_dec_cell(nc, pools, nkx, lhsT_fn, xrhs, xn_ks, xn_rhs, hb,
                   rzb_s, bin_s, bhn_s):
    """One decoder GRU cell.

    nkx: number of x k-tiles in the r/z matmul (0 when the x and hidden
    weights were pre-summed on the host).  lhsT_fn(part, j, a) -> weight AP
    for part in {"rz","xn","hn"}.  xrhs(j) -> rhs AP for rz x k-tile j.
    xn_ks/xn_rhs: k-tile count and rhs for the n-gate input projection.
    Hidden = (hf, hb).  Returns (new hf, new hb).
    """
    tmp = pools["tmp"]
    ps_rz, ps_ghn, ps_x = pools["ps_rz"], pools["ps_ghn"], pools["ps_gxn"]
    nk = nkx + KH
    p_rz = ps_rz.tile([128, 16, 128], F32, tag="prz")
    for j in range(nk):
        rhs = xrhs(j) if j < nkx else hb[:, j - nkx, :]
        for a in range(16):
            nc.tensor.matmul(p_rz[:, a, :], lhsT_fn("rz", j, a), rhs,
                             start=(j == 0 and a % 4 == 0),
                             stop=(j == nk - 1 and a % 4 == 3))
    rz_sb = tmp.tile([128, 16, 128], FP16, tag="rz")
    for a in range(16):
        nc.scalar.activation(rz_sb[:, a, :], p_rz[:, a, :], AF.Sigmoid,
                             bias=rzb_s[:, a:a + 1])
    p_gxn = ps_x.tile([128, 8, 128], F32, tag="pgxn")
    for j in range(xn_ks):
        for a in range(8):
            nc.tensor.matmul(p_gxn[:, a, :], lhsT_fn("xn", j, a), xn_rhs(j),
                             start=(j == 0 and a % 4 == 0),
                             stop=(j == xn_ks - 1 and a % 4 == 3))
    p_ghn = ps_ghn.tile([128, 8, 128], F32, tag="pghn")
    for a in range(8):
        for j in range(KH):
            nc.tensor.matmul(p_ghn[:, a, :], lhsT_fn("hn", j, a), hb[:, j, :],
                             start=(a % 4 == 0 and j == 0),
                             stop=(a % 4 == 3 and j == KH - 1))
    return _emit_chain(nc, pools, p_rz, p_ghn, hb, rz_sb,
                       ("psum", p_gxn), bhn_s, bin_s)


def build_program():
    nc = bacc.Bacc("TRN2", target_bir_lowering=False, debug=False,
                   num_devices=N_CORES)
    dp = nc.declare_dram_parameter
    obsT = dp("obsT", [2, T * BL], FP16, isOutput=False)
    WeT = dp("WeT", [2, E], FP16, isOutput=False)
    Wih1T = dp("Wih1T", [KE, 128, G], FP16, isOutput=False)
    Whh1T = dp("Whh1T", [KH, 128, G], FP16, isOutput=False)
    Wih2T = dp("Wih2T", [KH, 128, G], FP16, isOutput=False)
    Whh2T = dp("Whh2T", [KH, 128, G], FP16, isOutput=False)
    Wd1T = dp("Wd1T", [KE + KH, 128, G], FP16, isOutput=False)
    Wd2T = dp("Wd2T", [KH, 128, 4096], FP16, isOutput=False)
    WedT = dp("WedT", [2, E], FP16, isOutput=False)
    WoutT = dp("WoutT", [KH, 128, 2], FP16, isOutput=False)
    be_s = dp("be_s", [128, KE], F32, isOutput=False)
    gxb1 = dp("gxb1", [128, 24], F32, isOutput=False)
    gxb2 = dp("gxb2", [128, 24], F32, isOutput=False)
    bhn1 = dp("bhn1", [128, KH], F32, isOutput=False)
    bhn2 = dp("bhn2", [128, KH], F32, isOutput=False)
    d1_rzb = dp("d1_rzb", [128, 16], F32, isOutput=False)
    d1_bin = dp("d1_bin", [128, KH], F32, isOutput=False)
    d1_bhn = dp("d1_bhn", [128, KH], F32, isOutput=False)
    d2_rzb = dp("d2_rzb", [128, 16], F32, isOutput=False)
    d2_bin = dp("d2_bin", [128, KH], F32, isOutput=False)
    d2_bhn = dp("d2_bhn", [128, KH], F32, isOutput=False)
    bed_s = dp("bed_s", [128, KE], F32, isOutput=False)
    bout_s = dp("bout_s", [2, 1], F32, isOutput=False)
    preds = dp("preds", [2, PRED, BL], F32, isOutput=True)

    gx1 = nc.dram_tensor("gx1", [T, 24, 128, BL], F32)
    ys1 = nc.dram_tensor("ys1", [T, KH, 128, BL], FP16)
    gx2 = nc.dram_tensor("gx2", [T, 24, 128, BL], F32)

    with tile.TileContext(nc) as tc:
        with tc.tile_pool(name="const", bufs=1) as constp, \
             tc.tile_pool(name="hbp", bufs=2) as hbp, \
             tc.tile_pool(name="outp", bufs=2) as outp:
            def cload(name, ap, shape, dtype=F32):
                t = constp.tile(shape, dtype, tag=name)
                nc.sync.dma_start(out=t[:], in_=ap)
                return t
            gxb1_s = cload("gxb1", gxb1[:], [128, 24])
            gxb2_s = cload("gxb2", gxb2[:], [128, 24])
            be_c = cload("be", be_s[:], [128, KE])
            bhn1_s = cload("bhn1", bhn1[:], [128, KH])
            bhn2_s = cload("bhn2", bhn2[:], [128, KH])
            bout_c = cload("bout", bout_s[:], [2, 1])

            pools = {"hb": hbp}

            # ---------------- phase A: emb + gx1 ----------------
            with tc.tile_pool(name="wA", bufs=1) as wA, \
                 tc.tile_pool(name="sA", bufs=1) as sA, \
                 tc.tile_pool(name="embp", bufs=2) as embp, \
                 tc.tile_pool(name="gxoA", bufs=4) as gxoA, \
                 tc.tile_pool(name="psE", bufs=1, space="PSUM") as psE, \
                 tc.tile_pool(name="psGA", bufs=4, space="PSUM") as psGA:
                weT_s = wA.tile([2, E], FP16)
                nc.sync.dma_start(out=weT_s[:], in_=WeT[:])
                wih1_s = wA.tile([128, KE, G], FP16)
                for j in range(KE):
                    nc.sync.dma_start(out=wih1_s[:, j, :], in_=Wih1T[j])
                obs_s = sA.tile([2, T * BL], FP16)
                nc.sync.dma_start(out=obs_s[:], in_=obsT[:])

                def rhs_emb(c):
                    pe = psE.tile([128, KE, 512], F32, tag="pemb")
                    for et in range(KE):
                        nc.tensor.matmul(
                            pe[:, et, :], weT_s[:, et * 128:(et + 1) * 128],
                            obs_s[:, c * 512:(c + 1) * 512],
                            start=True, stop=True)
                    embb = embp.tile([128, KE, 512], FP16, tag="emb")
                    for et in range(KE):
                        nc.scalar.activation(embb[:, et, :], pe[:, et, :],
                                             AF.Tanh, bias=be_c[:, et:et + 1])
                    return [embb[:, et, :] for et in range(KE)]
                _emit_gx_phase(nc, tc, wih1_s, KE, rhs_emb, gx1, gxb1_s,
                               {"ps_gx": psGA, "gxo": gxoA})

            # ---------------- phase B: enc1 scan ----------------
            hb = hbp.tile([128, KH, 128], FP16, tag="hb")
            nc.vector.memset(hb[:], 0.0)
            with tc.tile_pool(name="wB", bufs=1) as wB, \
                 tc.tile_pool(name="gxB", bufs=5) as gxB, \
                 tc.tile_pool(name="tmpB", bufs=2) as tmpB, \
                 tc.tile_pool(name="psRZ", bufs=1, space="PSUM") as psRZ, \
                 tc.tile_pool(name="psGH", bufs=2, space="PSUM") as psGH:
                whh1_s = wB.tile([128, KH, G], FP16)
                for j in range(KH):
                    nc.sync.dma_start(out=whh1_s[:, j, :], in_=Whh1T[j])
                pls = dict(pools, gx=gxB, tmp=tmpB, ps_rz=psRZ, ps_ghn=psGH)
                hb, _ = _emit_enc_scan(nc, tc, pls, whh1_s, gx1, bhn1_s,
                                       hb, ys_dram=ys1)

            # ---------------- phase C: gx2 ----------------
            with tc.tile_pool(name="wC", bufs=1) as wC, \
                 tc.tile_pool(name="ysC", bufs=2) as ysC, \
                 tc.tile_pool(name="gxoC", bufs=4) as gxoC, \
                 tc.tile_pool(name="psGC", bufs=8, space="PSUM") as psGC:
                wih2_s = wC.tile([128, KH, G], FP16)
                for j in range(KH):
                    nc.sync.dma_start(out=wih2_s[:, j, :], in_=Wih2T[j])

                def rhs_ys(c):
                    ysr = ysC.tile([128, KH, 4, 128], FP16, tag="ysr")
                    for j in range(KH):
                        nc.sync.dma_start(
                            out=ysr[:, j],
                            in_=ys1[4 * c:4 * c + 4, j].rearrange(
                                "t p b -> p t b"))
                    return [ysr[:, j].rearrange("p t b -> p (t b)")
                            for j in range(KH)]
                _emit_gx_phase(nc, tc, wih2_s, KH, rhs_ys, gx2, gxb2_s,
                               {"ps_gx": psGC, "gxo": gxoC})

            # ---------------- phase D: enc2 scan (+ out_loc0 at t=98) -------
            with tc.tile_pool(name="wD", bufs=1) as wD, \
                 tc.tile_pool(name="gxD", bufs=5) as gxD, \
                 tc.tile_pool(name="tmpD", bufs=2) as tmpD, \
                 tc.tile_pool(name="psRZD", bufs=1, space="PSUM") as psRZD, \
                 tc.tile_pool(name="psGHD", bufs=2, space="PSUM") as psGHD:
                whh2_s = wD.tile([128, KH, G], FP16)
                for j in range(KH):
                    nc.sync.dma_start(out=whh2_s[:, j, :], in_=Whh2T[j])
                wout_s = wD.tile([128, KH, 2], FP16)
                nc.sync.dma_start(out=wout_s[:],
                                  in_=WoutT.ap().rearrange("j p m -> p j m"))
                pls = dict(pools, gx=gxD, tmp=tmpD, ps_rz=psRZD, ps_ghn=psGHD)
                hb, out_res = _emit_enc_scan(
                    nc, tc, pls, whh2_s, gx2, bhn2_s, hb,
                    out98=(wout_s, bout_c, outp))
                outf, outb = out_res

            # ---------------- phase E: decoder ----------------
            with tc.tile_pool(name="wE", bufs=1) as wE, \
                 tc.tile_pool(name="dembp", bufs=2) as dembp, \
                 tc.tile_pool(name="tmpE", bufs=1) as tmpE, \
                 tc.tile_pool(name="psRZE", bufs=1, space="PSUM") as psRZE, \
                 tc.tile_pool(name="psGHE", bufs=1, space="PSUM") as psGHE, \
                 tc.tile_pool(name="psXE", bufs=1, space="PSUM") as psXE:
                wd1_s = wE.tile([128, KE + KH, G], FP16)
                for j in range(KE + KH):
                    nc.sync.dma_start(out=wd1_s[:, j, :], in_=Wd1T[j])
                wd2_s = wE.tile([128, KH, 4096], FP16)
                for j in range(KH):
                    nc.sync.dma_start(out=wd2_s[:, j, :], in_=Wd2T[j])
                wed_s = wE.tile([2, E], FP16)
                nc.sync.dma_start(out=wed_s[:], in_=WedT[:])
                wout2_s = wE.tile([128, KH, 2], FP16)
                nc.sync.dma_start(out=wout2_s[:],
                                  in_=WoutT.ap().rearrange("j p m -> p j m"))
                rzb1_s = wE.tile([128, 16], F32)
                nc.sync.dma_start(out=rzb1_s[:], in_=d1_rzb[:])
                bin1_s = wE.tile([128, KH], F32)
                nc.sync.dma_start(out=bin1_s[:], in_=d1_bin[:])
                bhnd1_s = wE.tile([128, KH], F32)
                nc.sync.dma_start(out=bhnd1_s[:], in_=d1_bhn[:])
                rzb2_s = wE.tile([128, 16], F32)
                nc.sync.dma_start(out=rzb2_s[:], in_=d2_rzb[:])
                bin2_s = wE.tile([128, KH], F32)
                nc.sync.dma_start(out=bin2_s[:], in_=d2_bin[:])
                bhnd2_s = wE.tile([128, KH], F32)
                nc.sync.dma_start(out=bhnd2_s[:], in_=d2_bhn[:])
                bed_c = wE.tile([128, KE], F32)
                nc.sync.dma_start(out=bed_c[:], in_=bed_s[:])

                pls = dict(pools, tmp=tmpE, ps_rz=psRZE, ps_ghn=psGHE,
                           ps_gxn=psXE)
                for t in range(PRED):
                    p_de = psXE.tile([128, KE, 128], F32, tag="pgxn")
                    for et in range(KE):
                        nc.tensor.matmul(
                            p_de[:, et, :], wed_s[:, et * 128:(et + 1) * 128],
                            outb[:], start=(et == 0), stop=(et == KE - 1))
                    demb = dembp.tile([128, KE, 128], FP16, tag="demb")
                    for et in range(KE):
                        nc.scalar.activation(demb[:, et, :], p_de[:, et, :],
                                             AF.Tanh, bias=bed_c[:, et:et + 1])

                    def l1h(part, j, a):
                        if part == "hn":
                            return wd1_s[:, KE + j, (16 + a) * 128:(17 + a) * 128]
                        m = a if part == "rz" else 16 + a
                        return wd1_s[:, j, m * 128:(m + 1) * 128]
                    dembr = (lambda dd: lambda j: dd[:, j, :])(demb)
                    h1b = _emit_dec_cell(
                        nc, pls, KE, l1h, dembr, KE, dembr, hb,
                        rzb1_s, bin1_s, bhnd1_s)

                    def l2(part, j, a):
                        off = {"rz": a * 128, "xn": 2048 + a * 128,
                               "hn": 3072 + a * 128}[part]
                        return wd2_s[:, j, off:off + 128]
                    h1r = (lambda hh: lambda j: hh[:, j, :])(h1b)
                    hb = _emit_dec_cell(
                        nc, pls, 0, l2, None, KH, h1r, h1b,
                        rzb2_s, bin2_s, bhnd2_s)
                    outf, outb = _emit_out_block(nc, wout2_s, bout_c, hb,
                                                 psXE, "pgxn", outp, preds, t)
    nc.compile()
    return nc


# ----------------------------------------------------------------------------
# host side
# ----------------------------------------------------------------------------

def _tiles(w):
    """(G, fin) weight -> transposed k-tiles (fin/128, 128, G) bf16."""
    wt = np.ascontiguousarray(w.T)
    return wt.reshape(-1, 128, w.shape[0]).astype(np.float16)


def _cols(v):
    """(n*128,) bias -> (128, n) f32 with [p, j] = v[j*128+p]."""
    return np.ascontiguousarray(v.reshape(-1, 128).T.astype(np.float32))


def kernel(**inputs):
    ins = {k: np.asarray(v, np.float32) for k, v in inputs.items()}
    if "nc" not in _CACHE:
        _CACHE["nc"] = build_program()
    nc = _CACHE["nc"]

    w = {}
    w["WeT"] = np.ascontiguousarray(ins["We"].T).astype(np.float16)
    w["WedT"] = np.ascontiguousarray(ins["Wed"].T).astype(np.float16)
    P24 = [0, 1, 8, 9, 2, 3, 10, 11, 4, 5, 12, 13, 6, 7, 14, 15] + \
        list(range(16, 24))

    def _perm(wt):
        kk = wt.shape[0]
        return np.ascontiguousarray(
            wt.reshape(kk, 128, 24, 128)[:, :, P24].reshape(kk, 128, G))
    w["Wih1T"] = _perm(_tiles(ins["enc1_Wih"]))
    w["Whh1T"] = _perm(_tiles(ins["enc1_Whh"]))
    w["Wih2T"] = _perm(_tiles(ins["enc2_Wih"]))
    w["Whh2T"] = _perm(_tiles(ins["enc2_Whh"]))
    w["Wd1T"] = np.concatenate(
        [_tiles(ins["dec1_Wih"]), _tiles(ins["dec1_Whh"])], axis=0)
    wi, wh = ins["dec2_Wih"], ins["dec2_Whh"]
    wd2 = np.concatenate(
        [np.ascontiguousarray((wi[:2 * H] + wh[:2 * H]).T),
         np.ascontiguousarray(wi[2 * H:].T),
         np.ascontiguousarray(wh[2 * H:].T)], axis=1)  # (H, 4096)
    w["Wd2T"] = wd2.reshape(KH, 128, 4096).astype(np.float16)
    w["WoutT"] = np.ascontiguousarray(ins["Wout"].T).reshape(
        KH, 128, 2).astype(np.float16)
    w["be_s"] = _cols(ins["be"])
    w["bed_s"] = _cols(ins["bed"])
    w["bout_s"] = ins["bout"].reshape(2, 1).astype(np.float32)
    for pre, gq, bq in (("enc1", "gxb1", "bhn1"), ("enc2", "gxb2", "bhn2")):
        bih, bhh = ins[pre + "_bih"], ins[pre + "_bhh"]
        w[gq] = _cols(np.concatenate(
            [bih[:2 * H] + bhh[:2 * H], bih[2 * H:]]))[:, P24]
        w[gq] = np.ascontiguousarray(w[gq])
        w[bq] = _cols(bhh[2 * H:])
    for pre, tag in (("dec1", "d1"), ("dec2", "d2")):
        bih, bhh = ins[pre + "_bih"], ins[pre + "_bhh"]
        w[tag + "_rzb"] = _cols(bih[:2 * H] + bhh[:2 * H])
        w[tag + "_bin"] = _cols(bih[2 * H:])
        w[tag + "_bhn"] = _cols(bhh[2 * H:])

    obs = ins["obs"]
    in_maps = []
    for c in range(N_CORES):
        m = dict(w)
        ob = obs[c * BL:(c + 1) * BL]                  # (BL, T, 2)
        m["obsT"] = np.ascontiguousarray(
            ob.transpose(2, 1, 0)).reshape(2, T * BL).astype(np.float16)
        in_maps.append(m)

    _CACHE["in_maps"] = in_maps
    res = run_bass_kernel_spmd(nc, in_maps, list(range(N_CORES)))
    outs = []
    for c in range(N_CORES):
        p = res.results[c]["preds"]                    # (2, PRED, BL)
        outs.append(np.ascontiguousarray(p.transpose(2, 1, 0)))
    return np.concatenate(outs, axis=0).astype(np.float32)



# revision 2
# speedup vs baseline: 1.8557x; 1.8557x over previous
"""Trainium2 Bass kernel for the GRU encoder-decoder model.

Model (see harness reference): B=1024, T=100, PRED=30, E=512, H=1024, IN=2.
  emb = tanh(obs @ We.T + be)                      (B,T,512)
  enc1 = GRU(emb), enc2 = GRU(enc1, h0=h_enc1)     hidden 1024
  out0 = enc2[:,-2] @ Wout.T + bout
  30-step autoregressive decoder with two GRU cells sharing one hidden.

Strategy: data-parallel over batch on 8 cores (128 rows/core).  All compute is
done in a feature-on-partition ("transposed") layout: SBUF tiles are
[128 partitions = feature dim chunk, free = (tile, batch)].  Weights are
pre-transposed on the host and used as the stationary matmul operand in bf16;
PSUM accumulates in fp32.  The time-invariant input projections (gx = x@Wih.T
+ biases) for both encoder GRUs are precomputed in large N=512 matmul phases
and spilled to DRAM as bf16; the sequential scans then only run the h@Whh.T
recurrence plus the gate math.  The decoder is fused (autoregressive).  The
dec2 cell reads the same vector (h1) for both input and hidden, so its r/z
input+hidden weights are summed on the host, saving a third of its matmuls.
"""

import numpy as np
import ml_dtypes

import concourse.bass as bass
import concourse.mybir as mybir
import concourse.tile as tile
from concourse import bacc
from concourse.bass_utils import run_bass_kernel_spmd

F32 = mybir.dt.float32
F8 = mybir.dt.float8e4
BF16 = mybir.dt.bfloat16
FP16 = mybir.dt.float16
AF = mybir.ActivationFunctionType
ALU = mybir.AluOpType

N_CORES = 8
B, T, PRED = 1024, 100, 30
E, H, IN = 512, 1024, 2
BL = B // N_CORES          # 128 batch rows per core
G = 3 * H                  # 3072 stacked gate rows
KH = H // 128              # 8 hidden k-tiles
KE = E // 128              # 4 embedding k-tiles
NT = T * BL // 512         # 25 n-chunks of 512 in the gx phases
CHUNKS = [(0, 2), (2, 4), (4, 6), (6, 8)]  # h-tile chunks for the gate chain

E4 = ml_dtypes.float8_e4m3
WS8, ASC8 = 64.0, 16.0
ISC8 = 1.0 / (WS8 * ASC8)

_CACHE = {}


# ----------------------------------------------------------------------------
# device program
# ----------------------------------------------------------------------------

def _emit_gx_phase(nc, tc, wk, nk, rhs_fn, gx_dram, gxb_s, pools):
    """One n-chunk loop computing gx' = Wih.T-tiles @ rhs (+bias) -> DRAM.

    wk: SBUF weight tile [128, nk, G]; rhs_fn(c, pools) -> rhs tile [128, nk, 512]
    """
    ps_gx = pools["ps_gx"]
    gxop = pools["gxo"]
    DRM = mybir.MatmulPerfMode.DoubleRow
    for c in range(NT):
        rhs = rhs_fn(c)   # fp8 tile [128, nk, 512] (x16-scaled)
        for a in range(24):
            pg = ps_gx.tile([128, 512], F32, tag="pgx")
            for p in range(nk // 2):
                nc.tensor.matmul(
                    pg[:], wk[:, 2 * p:2 * p + 2, a * 128:(a + 1) * 128],
                    rhs[:, 2 * p:2 * p + 2, :],
                    start=(p == 0), stop=(p == nk // 2 - 1), perf_mode=DRM)
            gxo = gxop.tile([128, 4, 128], FP16, tag="gxo")
            nc.scalar.activation(
                gxo.rearrange("p t b -> p (t b)"), pg[:], AF.Identity,
                bias=gxb_s[:, a:a + 1], scale=ISC8)
            nc.sync.dma_start(
                out=gx_dram[4 * c:4 * c + 4, a].rearrange("t p b -> p t b"),
                in_=gxo[:])


def _emit_out_block(nc, wout_s, bout_s, hb, ps_pool, ps_tag, outp,
                    preds=None, t=None):
    """outT = h @ Wout.T + bout -> ([2,128] f32, [2,128] bf16)."""
    po = ps_pool.tile([2, 128], F32, tag=ps_tag)
    for j in range(KH):
        nc.tensor.matmul(po[:], wout_s[:, j, :], hb[:, j, :],
                         start=(j == 0), stop=(j == KH - 1))
    outf = outp.tile([2, 128], F32, tag="outf")
    outb = outp.tile([2, 128], FP16, tag="outb")
    nc.scalar.activation(outb[:], po[:], AF.Identity, bias=bout_s[:, 0:1])
    nc.vector.tensor_scalar_add(outf[:], po[:], bout_s[:, 0:1])
    if preds is not None:
        nc.sync.dma_start(out=preds[:, t, :], in_=outf[:])
    return outf, outb


def _emit_chain(nc, pools, p_rz, p_ghn, hb, rz_sb, gxn_src, bhn_s, bin_s,
                perm=False):
    """Gate math after the matmuls: returns the new fp16 hidden state.

    rz_sb: [128,16,128] fp16 sigmoid(r,z).  With perm=True the r/z tiles are
    bank-interleaved [r0 r1 z0 z1 | r2 r3 z2 z3 | ...]; otherwise [r*8, z*8].
    gxn_src: either ("sbuf", gx_tile) with n-part at tiles 16:24 (biases
    folded) or ("psum", p_gxn) requiring the bin_s bias.
    """
    tmp = pools["tmp"]
    hbp = pools["hb"]
    t1 = tmp.tile([128, 8, 128], FP16, tag="t1")
    t2 = tmp.tile([128, 8, 128], FP16, tag="t2")
    nn = tmp.tile([128, 8, 128], FP16, tag="nn")
    d = tmp.tile([128, 8, 128], FP16, tag="d")
    hz = tmp.tile([128, 8, 128], FP16, tag="hz")
    nhb = hbp.tile([128, 8, 128], FP16, tag="hb")
    kind, gxn = gxn_src
    for lo, hi in CHUNKS:
        for j in range(lo, hi):
            rj = 4 * (j // 2) + (j % 2) if perm else j
            nc.vector.scalar_tensor_tensor(
                t1[:, j, :], p_ghn[:, j, :], bhn_s[:, j:j + 1], rz_sb[:, rj, :],
                op0=ALU.add, op1=ALU.mult)
            if kind == "sbuf":
                nc.vector.tensor_add(t2[:, j, :], t1[:, j, :], gxn[:, 16 + j, :])
            else:
                nc.vector.scalar_tensor_tensor(
                    t2[:, j, :], gxn[:, j, :], bin_s[:, j:j + 1], t1[:, j, :],
                    op0=ALU.add, op1=ALU.add)
        sl = slice(lo, hi)
        zsl = slice(2 * lo + 2, 2 * lo + 4) if perm else slice(8 + lo, 8 + hi)
        nc.scalar.activation(nn[:, sl, :], t2[:, sl, :], AF.Tanh)
        nc.vector.tensor_sub(d[:, sl, :], hb[:, sl, :], nn[:, sl, :])
        nc.vector.tensor_mul(hz[:, sl, :], d[:, sl, :], rz_sb[:, zsl, :])
        nc.vector.tensor_add(nhb[:, sl, :], hz[:, sl, :], nn[:, sl, :])
    return nhb


def _emit_enc_scan(nc, tc, pools, whh_s, gx_dram, bhn_s, hb,
                   ys_dram=None, out98=None):
    """Encoder scan over T steps.  Returns (hf, hb) and optionally (outf, outb)
    captured at t = T-2 via out98 = (wout_s, bout_s, outp)."""
    gxp = pools["gx"]
    tmp = pools["tmp"]
    ps_rz, ps_ghn = pools["ps_rz"], pools["ps_ghn"]
    PF = 4
    pend = []
    for t in range(min(PF, T)):
        gxt = gxp.tile([128, 24, 128], FP16, tag="gxt")
        nc.sync.dma_start(out=gxt[:], in_=gx_dram[t].rearrange("a p b -> p a b"))
        pend.append(gxt)
    out_res = None
    for t in range(T):
        gxt = pend.pop(0)
        p_rz = ps_rz.tile([128, 16, 128], F32, tag="prz")
        rzs = tmp.tile([128, 16, 128], F32, tag="rzs")
        rz_sb = tmp.tile([128, 16, 128], FP16, tag="rz")
        # Bank-outer sweep: each PSUM bank (4 r/z tiles, bank-interleaved
        # [r0 r1 z0 z1 | ...]) is one accumulation group (start clears the
        # whole bank), and its bias-add + sigmoid issue as soon as the bank
        # finishes, overlapping the remaining banks' matmuls.
        for bk in range(4):
            for j in range(KH):           # k-inner: consume h chunks early
                for ai in range(4):
                    a = 4 * bk + ai
                    nc.tensor.matmul(
                        p_rz[:, a, :], whh_s[:, j, a * 128:(a + 1) * 128],
                        hb[:, j, :], start=(j == 0 and ai == 0),
                        stop=(j == KH - 1 and ai == 3))
            sl = slice(4 * bk, 4 * bk + 4)
            nc.vector.tensor_add(rzs[:, sl, :], p_rz[:, sl, :], gxt[:, sl, :])
            nc.scalar.activation(rz_sb[:, sl, :], rzs[:, sl, :], AF.Sigmoid)
        p_ghn = ps_ghn.tile([128, 8, 128], F32, tag="pghn")
        for a in range(8):                # m-outer: complete tiles early
            for j in range(KH):
                nc.tensor.matmul(
                    p_ghn[:, a, :], whh_s[:, j, (16 + a) * 128:(17 + a) * 128],
                    hb[:, j, :], start=(a % 4 == 0 and j == 0),
                    stop=(a % 4 == 3 and j == KH - 1))
        hb = _emit_chain(nc, pools, p_rz, p_ghn, hb, rz_sb,
                         ("sbuf", gxt), bhn_s, None, perm=True)
        if ys_dram is not None:
            ysb = pools["tmp"].tile([128, KH, 128], F8, tag="ys8")
            nc.vector.tensor_scalar_mul(ysb[:], hb[:], ASC8)
            nc.sync.dma_start(
                out=ys_dram[t].rearrange("j p b -> p j b"), in_=ysb[:])
        if out98 is not None and t == T - 2:
            wout_s, bout_s, outp = out98
            out_res = _emit_out_block(nc, wout_s, bout_s, hb, ps_ghn, "pghn",
                                      outp)
        if t + PF < T:
            gxt2 = gxp.tile([128, 24, 128], FP16, tag="gxt")
            nc.sync.dma_start(out=gxt2[:],
                              in_=gx_dram[t + PF].rearrange("a p b -> p a b"))
            pend.append(gxt2)
    return hb, out_res


def _emit_dec_cell(nc, pools, nkx, lhsT_fn, xrhs, xn_ks, xn_rhs, hb,
                   rzb_s, bin_s, bhn_s):
    """One decoder GRU cell.

    nkx: number of x k-tiles in the r/z matmul (0 when the x and hidden
    weights were pre-summed on the host).  lhsT_fn(part, j, a) -> weight AP
    for part in {"rz","xn","hn"}.  xrhs(j) -> rhs AP for rz x k-tile j.
    xn_ks/xn_rhs: k-tile count and rhs for the n-gate input projection.
    Hidden = (hf, hb).  Returns (new hf, new hb).
    """
    tmp = pools["tmp"]
    ps_rz, ps_ghn, ps_x = pools["ps_rz"], pools["ps_ghn"], pools["ps_gxn"]
    nk = nkx + KH
    p_rz = ps_rz.tile([128, 16, 128], F32, tag="prz")
    order = list(range(nkx, nk)) + list(range(nkx))
    for idx, j in enumerate(order):
        rhs = xrhs(j) if j < nkx else hb[:, j - nkx, :]
        for a in range(16):
            nc.tensor.matmul(p_rz[:, a, :], lhsT_fn("rz", j, a), rhs,
                             start=(idx == 0 and a % 4 == 0),
                             stop=(idx == nk - 1 and a % 4 == 3))
    rz_sb = tmp.tile([128, 16, 128], FP16, tag="rz")
    for a in range(16):
        nc.scalar.activation(rz_sb[:, a, :], p_rz[:, a, :], AF.Sigmoid,
                             bias=rzb_s[:, a:a + 1])
    p_ghn = ps_ghn.tile([128, 8, 128], F32, tag="pghn")
    for a in range(8):
        for j in range(KH):
            nc.tensor.matmul(p_ghn[:, a, :], lhsT_fn("hn", j, a), hb[:, j, :],
                             start=(a % 4 == 0 and j == 0),
                             stop=(a % 4 == 3 and j == KH - 1))
    p_gxn = ps_x.tile([128, 8, 128], F32, tag="pgxn")
    for j in range(xn_ks):
        for a in range(8):
            nc.tensor.matmul(p_gxn[:, a, :], lhsT_fn("xn", j, a), xn_rhs(j),
                             start=(j == 0 and a % 4 == 0),
                             stop=(j == xn_ks - 1 and a % 4 == 3))
    return _emit_chain(nc, pools, p_rz, p_ghn, hb, rz_sb,
                       ("psum", p_gxn), bhn_s, bin_s)


def build_program():
    nc = bacc.Bacc("TRN2", target_bir_lowering=False, debug=False,
                   num_devices=N_CORES)
    dp = nc.declare_dram_parameter
    obsT = dp("obsT", [2, T * BL], FP16, isOutput=False)
    WeT = dp("WeT", [2, E], FP16, isOutput=False)
    Wih1T = dp("Wih1T", [KE, 128, G], F8, isOutput=False)
    Whh1T = dp("Whh1T", [KH, 128, G], FP16, isOutput=False)
    Wih2T = dp("Wih2T", [KH, 128, G], F8, isOutput=False)
    Whh2T = dp("Whh2T", [KH, 128, G], FP16, isOutput=False)
    Wd1T = dp("Wd1T", [KE + KH, 128, G], FP16, isOutput=False)
    Wd2T = dp("Wd2T", [KH, 128, 4096], FP16, isOutput=False)
    WedT = dp("WedT", [2, E], FP16, isOutput=False)
    WoutT = dp("WoutT", [KH, 128, 2], FP16, isOutput=False)
    be_s = dp("be_s", [128, KE], F32, isOutput=False)
    gxb1 = dp("gxb1", [128, 24], F32, isOutput=False)
    gxb2 = dp("gxb2", [128, 24], F32, isOutput=False)
    bhn1 = dp("bhn1", [128, KH], F32, isOutput=False)
    bhn2 = dp("bhn2", [128, KH], F32, isOutput=False)
    d1_rzb = dp("d1_rzb", [128, 16], F32, isOutput=False)
    d1_bin = dp("d1_bin", [128, KH], F32, isOutput=False)
    d1_bhn = dp("d1_bhn", [128, KH], F32, isOutput=False)
    d2_rzb = dp("d2_rzb", [128, 16], F32, isOutput=False)
    d2_bin = dp("d2_bin", [128, KH], F32, isOutput=False)
    d2_bhn = dp("d2_bhn", [128, KH], F32, isOutput=False)
    bed_s = dp("bed_s", [128, KE], F32, isOutput=False)
    bout_s = dp("bout_s", [2, 1], F32, isOutput=False)
    preds = dp("preds", [2, PRED, BL], F32, isOutput=True)

    gx1 = nc.dram_tensor("gx1", [T, 24, 128, BL], FP16)
    ys1 = nc.dram_tensor("ys1", [T, KH, 128, BL], F8)
    gx2 = nc.dram_tensor("gx2", [T, 24, 128, BL], FP16)

    with tile.TileContext(nc) as tc:
        with tc.tile_pool(name="const", bufs=1) as constp, \
             tc.tile_pool(name="hbp", bufs=2) as hbp, \
             tc.tile_pool(name="outp", bufs=2) as outp:
            def cload(name, ap, shape, dtype=F32):
                t = constp.tile(shape, dtype, tag=name)
                nc.sync.dma_start(out=t[:], in_=ap)
                return t
            gxb1_s = cload("gxb1", gxb1[:], [128, 24])
            gxb2_s = cload("gxb2", gxb2[:], [128, 24])
            be_c = cload("be", be_s[:], [128, KE])
            bhn1_s = cload("bhn1", bhn1[:], [128, KH])
            bhn2_s = cload("bhn2", bhn2[:], [128, KH])
            bout_c = cload("bout", bout_s[:], [2, 1])

            pools = {"hb": hbp}

            # ---------------- phase A: emb + gx1 ----------------
            with tc.tile_pool(name="wA", bufs=1) as wA, \
                 tc.tile_pool(name="sA", bufs=1) as sA, \
                 tc.tile_pool(name="embp", bufs=2) as embp, \
                 tc.tile_pool(name="gxoA", bufs=4) as gxoA, \
                 tc.tile_pool(name="psE", bufs=1, space="PSUM") as psE, \
                 tc.tile_pool(name="psGA", bufs=4, space="PSUM") as psGA:
                weT_s = wA.tile([2, E], FP16)
                nc.sync.dma_start(out=weT_s[:], in_=WeT[:])
                wih1_s = wA.tile([128, KE, G], F8)
                for j in range(KE):
                    nc.sync.dma_start(out=wih1_s[:, j, :], in_=Wih1T[j])
                obs_s = sA.tile([2, T * BL], FP16)
                nc.sync.dma_start(out=obs_s[:], in_=obsT[:])

                def rhs_emb(c):
                    pe = psE.tile([128, KE, 512], F32, tag="pemb")
                    for et in range(KE):
                        nc.tensor.matmul(
                            pe[:, et, :], weT_s[:, et * 128:(et + 1) * 128],
                            obs_s[:, c * 512:(c + 1) * 512],
                            start=True, stop=True)
                    embb = embp.tile([128, KE, 512], FP16, tag="emb")
                    for et in range(KE):
                        nc.scalar.activation(embb[:, et, :], pe[:, et, :],
                                             AF.Tanh, bias=be_c[:, et:et + 1])
                    emb8 = embp.tile([128, KE, 512], F8, tag="emb8")
                    nc.vector.tensor_scalar_mul(emb8[:], embb[:], ASC8)
                    return emb8
                _emit_gx_phase(nc, tc, wih1_s, KE, rhs_emb, gx1, gxb1_s,
                               {"ps_gx": psGA, "gxo": gxoA})

            # ---------------- phase B: enc1 scan ----------------
            hb = hbp.tile([128, KH, 128], FP16, tag="hb")
            nc.vector.memset(hb[:], 0.0)
            with tc.tile_pool(name="wB", bufs=1) as wB, \
                 tc.tile_pool(name="gxB", bufs=5) as gxB, \
                 tc.tile_pool(name="tmpB", bufs=2) as tmpB, \
                 tc.tile_pool(name="psRZ", bufs=1, space="PSUM") as psRZ, \
                 tc.tile_pool(name="psGH", bufs=2, space="PSUM") as psGH:
                whh1_s = wB.tile([128, KH, G], FP16)
                for j in range(KH):
                    nc.sync.dma_start(out=whh1_s[:, j, :], in_=Whh1T[j])
                pls = dict(pools, gx=gxB, tmp=tmpB, ps_rz=psRZ, ps_ghn=psGH)
                hb, _ = _emit_enc_scan(nc, tc, pls, whh1_s, gx1, bhn1_s,
                                       hb, ys_dram=ys1)

            # ---------------- phase C: gx2 ----------------
            with tc.tile_pool(name="wC", bufs=1) as wC, \
                 tc.tile_pool(name="ysC", bufs=2) as ysC, \
                 tc.tile_pool(name="gxoC", bufs=4) as gxoC, \
                 tc.tile_pool(name="psGC", bufs=8, space="PSUM") as psGC:
                wih2_s = wC.tile([128, KH, G], F8)
                for j in range(KH):
                    nc.sync.dma_start(out=wih2_s[:, j, :], in_=Wih2T[j])

                def rhs_ys(c):
                    ysr = ysC.tile([128, KH, 4, 128], F8, tag="ysr")
                    for j in range(KH):
                        nc.sync.dma_start(
                            out=ysr[:, j],
                            in_=ys1[4 * c:4 * c + 4, j].rearrange(
                                "t p b -> p t b"))
                    return ysr.rearrange("p j t b -> p j (t b)")
                _emit_gx_phase(nc, tc, wih2_s, KH, rhs_ys, gx2, gxb2_s,
                               {"ps_gx": psGC, "gxo": gxoC})

            # ---------------- phase D: enc2 scan (+ out_loc0 at t=98) -------
            with tc.tile_pool(name="wD", bufs=1) as wD, \
                 tc.tile_pool(name="gxD", bufs=5) as gxD, \
                 tc.tile_pool(name="tmpD", bufs=2) as tmpD, \
                 tc.tile_pool(name="psRZD", bufs=1, space="PSUM") as psRZD, \
                 tc.tile_pool(name="psGHD", bufs=2, space="PSUM") as psGHD:
                whh2_s = wD.tile([128, KH, G], FP16)
                for j in range(KH):
                    nc.sync.dma_start(out=whh2_s[:, j, :], in_=Whh2T[j])
                wout_s = wD.tile([128, KH, 2], FP16)
                nc.sync.dma_start(out=wout_s[:],
                                  in_=WoutT.ap().rearrange("j p m -> p j m"))
                pls = dict(pools, gx=gxD, tmp=tmpD, ps_rz=psRZD, ps_ghn=psGHD)
                hb, out_res = _emit_enc_scan(
                    nc, tc, pls, whh2_s, gx2, bhn2_s, hb,
                    out98=(wout_s, bout_c, outp))
                outf, outb = out_res

            # ---------------- phase E: decoder ----------------
            with tc.tile_pool(name="wE", bufs=1) as wE, \
                 tc.tile_pool(name="dembp", bufs=2) as dembp, \
                 tc.tile_pool(name="tmpE", bufs=1) as tmpE, \
                 tc.tile_pool(name="psRZE", bufs=1, space="PSUM") as psRZE, \
                 tc.tile_pool(name="psGHE", bufs=1, space="PSUM") as psGHE, \
                 tc.tile_pool(name="psXE", bufs=1, space="PSUM") as psXE:
                wd1_s = wE.tile([128, KE + KH, G], FP16)
                for j in range(KE + KH):
                    nc.sync.dma_start(out=wd1_s[:, j, :], in_=Wd1T[j])
                wd2_s = wE.tile([128, KH, 4096], FP16)
                for j in range(KH):
                    nc.sync.dma_start(out=wd2_s[:, j, :], in_=Wd2T[j])
                wed_s = wE.tile([2, E], FP16)
                nc.sync.dma_start(out=wed_s[:], in_=WedT[:])
                wout2_s = wE.tile([128, KH, 2], FP16)
                nc.sync.dma_start(out=wout2_s[:],
                                  in_=WoutT.ap().rearrange("j p m -> p j m"))
                rzb1_s = wE.tile([128, 16], F32)
                nc.sync.dma_start(out=rzb1_s[:], in_=d1_rzb[:])
                bin1_s = wE.tile([128, KH], F32)
                nc.sync.dma_start(out=bin1_s[:], in_=d1_bin[:])
                bhnd1_s = wE.tile([128, KH], F32)
                nc.sync.dma_start(out=bhnd1_s[:], in_=d1_bhn[:])
                rzb2_s = wE.tile([128, 16], F32)
                nc.sync.dma_start(out=rzb2_s[:], in_=d2_rzb[:])
                bin2_s = wE.tile([128, KH], F32)
                nc.sync.dma_start(out=bin2_s[:], in_=d2_bin[:])
                bhnd2_s = wE.tile([128, KH], F32)
                nc.sync.dma_start(out=bhnd2_s[:], in_=d2_bhn[:])
                bed_c = wE.tile([128, KE], F32)
                nc.sync.dma_start(out=bed_c[:], in_=bed_s[:])

                pls = dict(pools, tmp=tmpE, ps_rz=psRZE, ps_ghn=psGHE,
                           ps_gxn=psXE)
                for t in range(PRED):
                    p_de = psXE.tile([128, KE, 128], F32, tag="pgxn")
                    for et in range(KE):
                        nc.tensor.matmul(
                            p_de[:, et, :], wed_s[:, et * 128:(et + 1) * 128],
                            outb[:], start=(et == 0), stop=(et == KE - 1))
                    demb = dembp.tile([128, KE, 128], FP16, tag="demb")
                    for et in range(KE):
                        nc.scalar.activation(demb[:, et, :], p_de[:, et, :],
                                             AF.Tanh, bias=bed_c[:, et:et + 1])

                    def l1h(part, j, a):
                        if part == "hn":
                            return wd1_s[:, KE + j, (16 + a) * 128:(17 + a) * 128]
                        m = a if part == "rz" else 16 + a
                        return wd1_s[:, j, m * 128:(m + 1) * 128]
                    dembr = (lambda dd: lambda j: dd[:, j, :])(demb)
                    h1b = _emit_dec_cell(
                        nc, pls, KE, l1h, dembr, KE, dembr, hb,
                        rzb1_s, bin1_s, bhnd1_s)

                    def l2(part, j, a):
                        off = {"rz": a * 128, "xn": 2048 + a * 128,
                               "hn": 3072 + a * 128}[part]
                        return wd2_s[:, j, off:off + 128]
                    h1r = (lambda hh: lambda j: hh[:, j, :])(h1b)
                    hb = _emit_dec_cell(
                        nc, pls, 0, l2, None, KH, h1r, h1b,
                        rzb2_s, bin2_s, bhnd2_s)
                    outf, outb = _emit_out_block(nc, wout2_s, bout_c, hb,
                                                 psXE, "pgxn", outp, preds, t)
    nc.compile()
    return nc


# ----------------------------------------------------------------------------
# host side
# ----------------------------------------------------------------------------

def _tiles(w):
    """(G, fin) weight -> transposed k-tiles (fin/128, 128, G) bf16."""
    wt = np.ascontiguousarray(w.T)
    return wt.reshape(-1, 128, w.shape[0]).astype(np.float16)


def _cols(v):
    """(n*128,) bias -> (128, n) f32 with [p, j] = v[j*128+p]."""
    return np.ascontiguousarray(v.reshape(-1, 128).T.astype(np.float32))


def kernel(**inputs):
    ins = {k: np.asarray(v, np.float32) for k, v in inputs.items()}
    if "nc" not in _CACHE:
        _CACHE["nc"] = build_program()
    nc = _CACHE["nc"]

    w = {}
    w["WeT"] = np.ascontiguousarray(ins["We"].T).astype(np.float16)
    w["WedT"] = np.ascontiguousarray(ins["Wed"].T).astype(np.float16)
    P24 = [0, 1, 8, 9, 2, 3, 10, 11, 4, 5, 12, 13, 6, 7, 14, 15] + \
        list(range(16, 24))

    def _perm(wt):
        kk = wt.shape[0]
        return np.ascontiguousarray(
            wt.reshape(kk, 128, 24, 128)[:, :, P24].reshape(kk, 128, G))
    w["Wih1T"] = np.ascontiguousarray(
        _perm(_tiles(ins["enc1_Wih"])).astype(np.float32) * WS8).astype(E4)
    w["Whh1T"] = _perm(_tiles(ins["enc1_Whh"]))
    w["Wih2T"] = np.ascontiguousarray(
        _perm(_tiles(ins["enc2_Wih"])).astype(np.float32) * WS8).astype(E4)
    w["Whh2T"] = _perm(_tiles(ins["enc2_Whh"]))
    w["Wd1T"] = np.concatenate(
        [_tiles(ins["dec1_Wih"]), _tiles(ins["dec1_Whh"])], axis=0)
    wi, wh = ins["dec2_Wih"], ins["dec2_Whh"]
    wd2 = np.concatenate(
        [np.ascontiguousarray((wi[:2 * H] + wh[:2 * H]).T),
         np.ascontiguousarray(wi[2 * H:].T),
         np.ascontiguousarray(wh[2 * H:].T)], axis=1)  # (H, 4096)
    w["Wd2T"] = wd2.reshape(KH, 128, 4096).astype(np.float16)
    w["WoutT"] = np.ascontiguousarray(ins["Wout"].T).reshape(
        KH, 128, 2).astype(np.float16)
    w["be_s"] = _cols(ins["be"])
    w["bed_s"] = _cols(ins["bed"])
    w["bout_s"] = ins["bout"].reshape(2, 1).astype(np.float32)
    for pre, gq, bq in (("enc1", "gxb1", "bhn1"), ("enc2", "gxb2", "bhn2")):
        bih, bhh = ins[pre + "_bih"], ins[pre + "_bhh"]
        w[gq] = _cols(np.concatenate(
            [bih[:2 * H] + bhh[:2 * H], bih[2 * H:]]))[:, P24]
        w[gq] = np.ascontiguousarray(w[gq])
        w[bq] = _cols(bhh[2 * H:])
    for pre, tag in (("dec1", "d1"), ("dec2", "d2")):
        bih, bhh = ins[pre + "_bih"], ins[pre + "_bhh"]
        w[tag + "_rzb"] = _cols(bih[:2 * H] + bhh[:2 * H])
        w[tag + "_bin"] = _cols(bih[2 * H:])
        w[tag + "_bhn"] = _cols(bhh[2 * H:])

    obs = ins["obs"]
    in_maps = []
    for c in range(N_CORES):
        m = dict(w)
        ob = obs[c * BL:(c + 1) * BL]                  # (BL, T, 2)
        m["obsT"] = np.ascontiguousarray(
            ob.transpose(2, 1, 0)).reshape(2, T * BL).astype(np.float16)
        in_maps.append(m)

    _CACHE["in_maps"] = in_maps
    res = run_bass_kernel_spmd(nc, in_maps, list(range(N_CORES)))
    outs = []
    for c in range(N_CORES):
        p = res.results[c]["preds"]                    # (2, PRED, BL)
        outs.append(np.ascontiguousarray(p.transpose(2, 1, 0)))
    return np.concatenate(outs, axis=0).astype(np.float32)

